# revision 20
# baseline (speedup 1.0000x reference)
"""Trainium2 Bass kernel for nn_GTN_72679436583060 (GTN message passing).

Math: with w-softmax over a singleton axis each GTConv is exactly 2*A, so

    out = 2 * rownorm(4*A@A + I) @ A
        = diag(8 / (4*rowsum(M) + 1)) @ (M@A + 0.25*A)   with M = A@A

Sharding: row-wise over 8 cores, A replicated. Per core (rows R = 256):
  GEMM1 (transposed):  MT = A^T @ (A_rows^T)        (2048 x 256), lhsT = A tiles
  deg:                 rowsum(M) via a ones-column matmul sharing GEMM2's lhsT
  GEMM2:               P = M @ A + 0.25*A_rows       (256 x 2048), lhsT = MT tiles
  epilogue:            out = P * (8 / (4*deg + 1))   per-row scale

All matmuls in bf16 (1 cycle/row on PE), fp32 PSUM accumulation, fp32 output.
GEMM1 runs k-outer so the PE tracks the streaming A DMA; all 16 output tile
groups fit in 8 PSUM banks via zero-writing "bank clear" matmuls (which also
warm up the PE HAM clock during the initial DMA window).
"""

import numpy as np

N = 2048
P = 128
NCORES = 8
R = N // NCORES        # 256 rows per core
KT = N // P            # 16 partition tiles
MT = R // P            # 2 row subtiles per core
FD = 512               # PSUM bank free dim (fp32)
NT2 = N // FD          # 4 GEMM2 n-tiles

_CACHE = {}


def _build_bass():
    from contextlib import ExitStack

    import concourse.bass as bass  # noqa: F401
    import concourse.mybir as mybir
    import concourse.tile as tile
    from concourse import bacc

    dt = mybir.dt
    fp32 = dt.float32
    bf16 = dt.bfloat16
    Alu = mybir.AluOpType

    nc = bacc.Bacc(None, target_bir_lowering=False)
    # a/art are shipped pre-tiled: row p of a_d holds A[k*128+p, :] for
    # k = 0..15 back to back, so a multi-k chunk DMA moves one long
    # contiguous run per partition (fewer, bigger descriptors — the DMA
    # queues are descriptor-rate limited, not byte limited).
    a_d = nc.dram_tensor("a", [KT // 2, P, 2 * N], bf16, kind="ExternalInput")
    art_d = nc.dram_tensor("art", [P, KT * R], bf16, kind="ExternalInput")
    ar_d = nc.dram_tensor("ar", [R, N], bf16, kind="ExternalInput")
    ones_d = nc.dram_tensor("ones", [P, 1], bf16, kind="ExternalInput")
    iq_d = nc.dram_tensor("iq", [P, P], bf16, kind="ExternalInput")
    out_d = nc.dram_tensor("out", [R, N], fp32, kind="ExternalOutput")

    with tile.TileContext(nc) as tc, ExitStack() as ctx:
        a_pool = ctx.enter_context(tc.tile_pool(name="a", bufs=1))
        art_pool = ctx.enter_context(tc.tile_pool(name="art", bufs=1))
        ar_pool = ctx.enter_context(tc.tile_pool(name="ar", bufs=MT))
        mt_pool = ctx.enter_context(tc.tile_pool(name="mt", bufs=KT))
        const_pool = ctx.enter_context(tc.tile_pool(name="const", bufs=1))
        outsb_pool = ctx.enter_context(tc.tile_pool(name="outsb", bufs=4))
        sc_pool = ctx.enter_context(tc.tile_pool(name="sc", bufs=4))

        zeros_t = const_pool.tile([P, FD], bf16, tag="zeros")
        nc.vector.memset(zeros_t[:], 0.0)

        # Stream A row-tiles (and the matching ART tiles) in k order; they
        # stay resident: GEMM1 uses A as lhsT, GEMM2 reuses it as rhs.
        # The tiny const/ar loads are issued last — they are only needed in
        # GEMM2, and issuing them first would delay the first k-sweep.
        # One HWDGE queue tops out well below the HBM limit (~117 GB/s
        # measured), so round-robin the big A-tile loads over three queues
        # (sync / scalar / vector) and split the first tiles in half for a
        # fast pipeline start. Cold constants ride the gpsimd SWDGE queue.
        a_big = a_pool.tile([P, KT * N], bf16, tag="a")
        art_big = art_pool.tile([P, KT * R], bf16, tag="art")
        a_tiles = [a_big[:, k * N:(k + 1) * N] for k in range(KT)]
        art_tiles = [art_big[:, k * R:(k + 1) * R] for k in range(KT)]
        # A ships as 8 dense 1MB groups of two k-tiles (8KB contiguous per
        # partition), ART packed as one dense region. Thin chunks first so
        # the early sweeps' dependencies stay small; gpsimd's SWDGE queue
        # carries the cold tail (last group, ar, consts) to offload the two
        # HWDGE queues.
        def dma_art(lo, hi, eng):
            eng.dma_start(art_big[:, lo * R:hi * R], art_d[:, lo * R:hi * R])

        def dma_ag(g, eng):
            eng.dma_start(a_big[:, g * 2 * N:(g + 1) * 2 * N], a_d[g, :, :])

        def dma_at(g, t, eng):
            lo = g * 2 * N + t * N
            eng.dma_start(a_big[:, lo:lo + N], a_d[g, :, t * N:(t + 1) * N])

        dma_art(0, 2, nc.sync)
        dma_art(2, 6, nc.scalar)
        dma_at(0, 0, nc.sync)
        dma_at(0, 1, nc.scalar)
        dma_at(1, 0, nc.sync)
        dma_at(1, 1, nc.scalar)
        dma_art(6, 11, nc.sync)
        dma_art(11, 16, nc.scalar)
        dma_ag(2, nc.sync)
        dma_ag(3, nc.scalar)
        dma_ag(4, nc.sync)
        dma_ag(5, nc.scalar)
        dma_ag(6, nc.sync)
        ar_tiles = []
        for m in range(MT):
            t = ar_pool.tile([P, N], bf16, tag="ar")
            nc.gpsimd.dma_start(t[:], ar_d[m * P:(m + 1) * P, :])
            ar_tiles.append(t)
        dma_ag(7, nc.gpsimd)
        ones_t = const_pool.tile([P, 1], bf16, tag="ones")
        nc.gpsimd.dma_start(ones_t[:], ones_d[:, :])
        iq_t = const_pool.tile([P, P], bf16, tag="iq")
        nc.gpsimd.dma_start(iq_t[:], iq_d[:, :])

        # ---- GEMM1: MT[j, r] = sum_k A[k, j] * A_rows[r, k], k-outer ----
        # Two j-groups share each PSUM bank. A start=True zero matmul per
        # bank clears it and sets every has_written bit, so all real
        # matmuls accumulate with start=False regardless of issue order.
        mt_tiles = [None] * KT
        # One shared PSUM pool (8 banks, one tag) for GEMM1 pair tiles,
        # GEMM2 output tiles and deg tiles: GEMM2's first allocations reuse
        # slots as soon as individual pair tiles are copied out, instead of
        # stalling on a whole-pool release at the phase boundary.
        with tc.tile_pool(name="psum", bufs=8, space="PSUM") as psum_pool:
            # Per-bank zero matmul: start=True clears the whole bank; writing
            # [255:257) spans both half-bank groups, so WAW deps keep every
            # real matmul ordered after the clear. Elements outside [255:257)
            # keep has_written unset, so each group's first real matmul
            # overwrites (= accumulate onto zero).
            pairs = []
            for b in range(KT // 2):
                ps = psum_pool.tile([P, FD], fp32, tag="bank", name=f"pair_{b}")
                nc.tensor.matmul(
                    ps[:, R - 1:R + 1], zeros_t[:, 0:P], zeros_t[:, 0:2],
                    start=True, stop=False, skip_group_check=True,
                )
                pairs.append(ps)
            for k in range(KT):
                for j in range(KT):
                    half = (j % 2) * R
                    nc.tensor.matmul(
                        pairs[j // 2][:, half:half + R],
                        a_tiles[k][:, j * P:(j + 1) * P],
                        art_tiles[k][:],
                        start=False, stop=(k == KT - 1),
                        skip_group_check=True,
                    )
            for j in range(KT):
                half = (j % 2) * R
                mt = mt_pool.tile([P, R], bf16, tag="mt")
                nc.vector.tensor_copy(mt[:], pairs[j // 2][:, half:half + R])
                mt_tiles[j] = mt

            # ---- GEMM2 + deg + epilogue ----
            # The 0.25*I matmul doubles as each bank's accumulation-group
            # starter (start=True clears the bank and seeds it with
            # 0.25*A_rows), so banks finish at their last j matmul.
            # m=0 runs j-outer (tracks the mt copies with no stall);
            # m=1 runs n-outer so its four banks complete staggered and the
            # final epilogues pipeline with PE instead of bunching at the end.
            def emit_epilogue(m, n, psum_tile, sca):
                ot = outsb_pool.tile([P, FD], fp32, tag="ot",
                                     name=f"ot_{m}_{n}")
                nc.vector.tensor_scalar(
                    out=ot[:], in0=psum_tile[:], scalar1=sca[:],
                    scalar2=None, op0=Alu.mult,
                )
                eng = nc.sync if n % 2 == 0 else nc.scalar
                eng.dma_start(
                    out_d[m * P:(m + 1) * P, n * FD:(n + 1) * FD], ot[:]
                )

            def emit_deg_scale(m, deg_ps):
                # scale = 8 / (4*deg + 1) == 1 / (0.5*deg + 0.125)
                t1 = sc_pool.tile([P, 1], fp32, tag="t1", name=f"t1_{m}")
                nc.vector.tensor_scalar(
                    out=t1[:], in0=deg_ps[:], scalar1=0.5, scalar2=0.125,
                    op0=Alu.mult, op1=Alu.add,
                )
                sca = sc_pool.tile([P, 1], fp32, tag="sca", name=f"sca_{m}")
                nc.vector.reciprocal(sca[:], t1[:])
                return sca

            # m = 0: j-outer
            m = 0
            outs_ps = [psum_pool.tile([P, FD], fp32, tag="bank",
                                      name=f"outps0_{i}") for i in range(NT2)]
            deg_full = psum_pool.tile([P, FD], fp32, tag="bank", name="deg_0")
            deg_ps = deg_full[:, 0:1]
            for n in range(NT2):
                nc.tensor.matmul(
                    outs_ps[n][:], iq_t[:],
                    ar_tiles[m][:, n * FD:(n + 1) * FD],
                    start=True, stop=False,
                )
            for j in range(KT):
                lhsT = mt_tiles[j][:, m * P:(m + 1) * P]
                for n in range(NT2):
                    nc.tensor.matmul(
                        outs_ps[n][:], lhsT,
                        a_tiles[j][:, n * FD:(n + 1) * FD],
                        start=False, stop=(j == KT - 1),
                    )
                nc.tensor.matmul(
                    deg_ps[:], lhsT, ones_t[:],
                    start=(j == 0), stop=(j == KT - 1),
                )
            sca = emit_deg_scale(m, deg_ps)
            for n in range(NT2):
                emit_epilogue(m, n, outs_ps[n], sca)

            # m = 1: n-outer, deg rides along with the n=0 bank
            m = 1
            deg_full = psum_pool.tile([P, FD], fp32, tag="bank", name="deg_1")
            deg_ps = deg_full[:, 0:1]
            sca = None
            for n in range(NT2):
                ops = psum_pool.tile([P, FD], fp32, tag="bank",
                                     name=f"outps1_{n}")
                nc.tensor.matmul(
                    ops[:], iq_t[:], ar_tiles[m][:, n * FD:(n + 1) * FD],
                    start=True, stop=False,
                )
                for j in range(KT):
                    lhsT = mt_tiles[j][:, m * P:(m + 1) * P]
                    nc.tensor.matmul(
                        ops[:], lhsT, a_tiles[j][:, n * FD:(n + 1) * FD],
                        start=False, stop=(j == KT - 1),
                    )
                    if n == 0:
                        nc.tensor.matmul(
                            deg_ps[:], lhsT, ones_t[:],
                            start=(j == 0), stop=(j == KT - 1),
                        )
                if n == 0:
                    sca = emit_deg_scale(m, deg_ps)
                emit_epilogue(m, n, ops, sca)
    nc.compile()
    return nc


def _get_nc():
    if "nc" not in _CACHE:
        _CACHE["nc"] = _build_bass()
    return _CACHE["nc"]


def _make_in_maps(A_f32):
    import ml_dtypes

    bf = ml_dtypes.bfloat16
    Ab = A_f32.astype(bf)
    ATb = np.ascontiguousarray(Ab.T)
    # A as 8 dense groups of two k-tiles: a_pk[g, p] = A[g*256+p] ++ A[g*256+128+p]
    a_pk = np.ascontiguousarray(
        Ab.reshape(KT // 2, 2, P, N).transpose(0, 2, 1, 3)
        .reshape(KT // 2, P, 2 * N))
    ones = np.ones((P, 1), dtype=bf)
    iq = (0.25 * np.eye(P, dtype=np.float32)).astype(bf)
    in_maps = []
    for c in range(NCORES):
        sl = slice(c * R, (c + 1) * R)
        art = ATb[:, sl].reshape(KT, P, R).transpose(1, 0, 2)
        in_maps.append({
            "a": a_pk,
            "art": np.ascontiguousarray(art).reshape(P, KT * R),
            "ar": np.ascontiguousarray(Ab[sl, :]),
            "ones": ones,
            "iq": iq,
        })
    return in_maps


def kernel(A, w1a=None, w1b=None, w2a=None, **_unused):
    # w1a/w1b/w2a only enter the reference through a softmax over a
    # singleton axis (== 1.0), so the output does not depend on them.
    from concourse.bass_utils import run_bass_kernel_spmd

    A = np.asarray(A, dtype=np.float32)
    assert A.shape == (N, N), A.shape
    nc = _get_nc()
    in_maps = _make_in_maps(A)
    res = run_bass_kernel_spmd(nc, in_maps, core_ids=list(range(NCORES)))
    out = np.concatenate(
        [res.results[c]["out"] for c in range(NCORES)], axis=0
    )
    return out[None].astype(np.float32)


# revision 21
# speedup vs baseline: 1.1436x; 1.1436x over previous
"""Trainium2 Bass kernel for nn_GTN_72679436583060 (GTN message passing).

Math: with w-softmax over a singleton axis each GTConv is exactly 2*A, so

    out = 2 * rownorm(4*A@A + I) @ A
        = diag(8 / (4*rowsum(M) + 1)) @ (M@A + 0.25*A)   with M = A@A

Sharding: row-wise over 8 cores, A replicated. Per core (rows R = 256):
  GEMM1 (transposed):  MT = A^T @ (A_rows^T)        (2048 x 256), lhsT = A tiles
  deg:                 rowsum(M) via a ones-column matmul sharing GEMM2's lhsT
  GEMM2:               P = M @ A + 0.25*A_rows       (256 x 2048), lhsT = MT tiles
  epilogue:            out = P * (8 / (4*deg + 1))   per-row scale

All matmuls in bf16 (1 cycle/row on PE), fp32 PSUM accumulation, fp32 output.
GEMM1 runs k-outer so the PE tracks the streaming A DMA; all 16 output tile
groups fit in 8 PSUM banks via zero-writing "bank clear" matmuls (which also
warm up the PE HAM clock during the initial DMA window).
"""

import numpy as np

N = 2048
P = 128
NCORES = 8
R = N // NCORES        # 256 rows per core
KT = N // P            # 16 partition tiles
MT = R // P            # 2 row subtiles per core
FD = 512               # PSUM bank free dim (fp32)
NT2 = N // FD          # 4 GEMM2 n-tiles

_CACHE = {}


def _build_bass():
    from contextlib import ExitStack

    import concourse.bass as bass  # noqa: F401
    import concourse.mybir as mybir
    import concourse.tile as tile
    from concourse import bacc

    dt = mybir.dt
    fp32 = dt.float32
    bf16 = dt.bfloat16
    Alu = mybir.AluOpType

    nc = bacc.Bacc(None, target_bir_lowering=False)
    # a/art are shipped pre-tiled: row p of a_d holds A[k*128+p, :] for
    # k = 0..15 back to back, so a multi-k chunk DMA moves one long
    # contiguous run per partition (fewer, bigger descriptors — the DMA
    # queues are descriptor-rate limited, not byte limited).
    a_d = nc.dram_tensor("a", [N, N], bf16, kind="ExternalInput")
    art_d = nc.dram_tensor("art", [N, R], bf16, kind="ExternalInput")
    ar_d = nc.dram_tensor("ar", [R, N], bf16, kind="ExternalInput")
    ones_d = nc.dram_tensor("ones", [P, 1], bf16, kind="ExternalInput")
    iq_d = nc.dram_tensor("iq", [P, P], bf16, kind="ExternalInput")
    out_d = nc.dram_tensor("out", [R, N], fp32, kind="ExternalOutput")

    with tile.TileContext(nc) as tc, ExitStack() as ctx:
        a_pool = ctx.enter_context(tc.tile_pool(name="a", bufs=KT))
        art_pool = ctx.enter_context(tc.tile_pool(name="art", bufs=KT))
        ar_pool = ctx.enter_context(tc.tile_pool(name="ar", bufs=MT))
        mt_pool = ctx.enter_context(tc.tile_pool(name="mt", bufs=KT))
        const_pool = ctx.enter_context(tc.tile_pool(name="const", bufs=1))
        outsb_pool = ctx.enter_context(tc.tile_pool(name="outsb", bufs=4))
        sc_pool = ctx.enter_context(tc.tile_pool(name="sc", bufs=4))

        zeros_t = const_pool.tile([P, FD], bf16, tag="zeros")
        nc.vector.memset(zeros_t[:], 0.0)

        # Stream A row-tiles (and the matching ART tiles) in k order; they
        # stay resident: GEMM1 uses A as lhsT, GEMM2 reuses it as rhs.
        # The tiny const/ar loads are issued last — they are only needed in
        # GEMM2, and issuing them first would delay the first k-sweep.
        # One HWDGE queue tops out well below the HBM limit (~117 GB/s
        # measured), so round-robin the big A-tile loads over three queues
        # (sync / scalar / vector) and split the first tiles in half for a
        # fast pipeline start. Cold constants ride the gpsimd SWDGE queue.
        a_tiles, art_tiles = [], []
        for k in range(KT):
            rt = art_pool.tile([P, R], bf16, tag="art")
            nc.sync.dma_start(rt[:], art_d[k * P:(k + 1) * P, :])
            art_tiles.append(rt)
            at = a_pool.tile([P, N], bf16, tag="a")
            eng = nc.sync if k % 2 == 0 else nc.scalar
            eng.dma_start(at[:], a_d[k * P:(k + 1) * P, :])
            a_tiles.append(at)
        ar_tiles = []
        for m in range(MT):
            t = ar_pool.tile([P, N], bf16, tag="ar")
            nc.sync.dma_start(t[:], ar_d[m * P:(m + 1) * P, :])
            ar_tiles.append(t)
        ones_t = const_pool.tile([P, 1], bf16, tag="ones")
        nc.sync.dma_start(ones_t[:], ones_d[:, :])
        iq_t = const_pool.tile([P, P], bf16, tag="iq")
        nc.sync.dma_start(iq_t[:], iq_d[:, :])

        # ---- GEMM1: MT[j, r] = sum_k A[k, j] * A_rows[r, k], k-outer ----
        # Two j-groups share each PSUM bank. A start=True zero matmul per
        # bank clears it and sets every has_written bit, so all real
        # matmuls accumulate with start=False regardless of issue order.
        mt_tiles = [None] * KT
        # One shared PSUM pool (8 banks, one tag) for GEMM1 pair tiles,
        # GEMM2 output tiles and deg tiles: GEMM2's first allocations reuse
        # slots as soon as individual pair tiles are copied out, instead of
        # stalling on a whole-pool release at the phase boundary.
        with tc.tile_pool(name="psum", bufs=8, space="PSUM") as psum_pool:
            # Per-bank zero matmul: start=True clears the whole bank; writing
            # [255:257) spans both half-bank groups, so WAW deps keep every
            # real matmul ordered after the clear. Elements outside [255:257)
            # keep has_written unset, so each group's first real matmul
            # overwrites (= accumulate onto zero).
            pairs = []
            for b in range(KT // 2):
                ps = psum_pool.tile([P, FD], fp32, tag="bank", name=f"pair_{b}")
                nc.tensor.matmul(
                    ps[:, R - 1:R + 1], zeros_t[:, 0:P], zeros_t[:, 0:2],
                    start=True, stop=False, skip_group_check=True,
                )
                pairs.append(ps)
            for k in range(KT):
                for j in range(KT):
                    half = (j % 2) * R
                    nc.tensor.matmul(
                        pairs[j // 2][:, half:half + R],
                        a_tiles[k][:, j * P:(j + 1) * P],
                        art_tiles[k][:],
                        start=False, stop=(k == KT - 1),
                        skip_group_check=True,
                    )
            for j in range(KT):
                half = (j % 2) * R
                mt = mt_pool.tile([P, R], bf16, tag="mt")
                nc.vector.tensor_copy(mt[:], pairs[j // 2][:, half:half + R])
                mt_tiles[j] = mt

            # ---- GEMM2 + deg + epilogue ----
            # The 0.25*I matmul doubles as each bank's accumulation-group
            # starter (start=True clears the bank and seeds it with
            # 0.25*A_rows), so banks finish at their last j matmul.
            # m=0 runs j-outer (tracks the mt copies with no stall);
            # m=1 runs n-outer so its four banks complete staggered and the
            # final epilogues pipeline with PE instead of bunching at the end.
            def emit_epilogue(m, n, psum_tile, sca):
                ot = outsb_pool.tile([P, FD], fp32, tag="ot",
                                     name=f"ot_{m}_{n}")
                nc.vector.tensor_scalar(
                    out=ot[:], in0=psum_tile[:], scalar1=sca[:],
                    scalar2=None, op0=Alu.mult,
                )
                eng = nc.sync if n % 2 == 0 else nc.scalar
                eng.dma_start(
                    out_d[m * P:(m + 1) * P, n * FD:(n + 1) * FD], ot[:]
                )

            def emit_deg_scale(m, deg_ps):
                # scale = 8 / (4*deg + 1) == 1 / (0.5*deg + 0.125)
                t1 = sc_pool.tile([P, 1], fp32, tag="t1", name=f"t1_{m}")
                nc.vector.tensor_scalar(
                    out=t1[:], in0=deg_ps[:], scalar1=0.5, scalar2=0.125,
                    op0=Alu.mult, op1=Alu.add,
                )
                sca = sc_pool.tile([P, 1], fp32, tag="sca", name=f"sca_{m}")
                nc.vector.reciprocal(sca[:], t1[:])
                return sca

            # m = 0: j-outer
            m = 0
            outs_ps = [psum_pool.tile([P, FD], fp32, tag="bank",
                                      name=f"outps0_{i}") for i in range(NT2)]
            deg_full = psum_pool.tile([P, FD], fp32, tag="bank", name="deg_0")
            deg_ps = deg_full[:, 0:1]
            for n in range(NT2):
                nc.tensor.matmul(
                    outs_ps[n][:], iq_t[:],
                    ar_tiles[m][:, n * FD:(n + 1) * FD],
                    start=True, stop=False,
                )
            for j in range(KT):
                lhsT = mt_tiles[j][:, m * P:(m + 1) * P]
                for n in range(NT2):
                    nc.tensor.matmul(
                        outs_ps[n][:], lhsT,
                        a_tiles[j][:, n * FD:(n + 1) * FD],
                        start=False, stop=(j == KT - 1),
                    )
                nc.tensor.matmul(
                    deg_ps[:], lhsT, ones_t[:],
                    start=(j == 0), stop=(j == KT - 1),
                )
            sca = emit_deg_scale(m, deg_ps)
            for n in range(NT2):
                emit_epilogue(m, n, outs_ps[n], sca)

            # m = 1: n-outer, deg rides along with the n=0 bank
            m = 1
            deg_full = psum_pool.tile([P, FD], fp32, tag="bank", name="deg_1")
            deg_ps = deg_full[:, 0:1]
            sca = None
            for n in range(NT2):
                ops = psum_pool.tile([P, FD], fp32, tag="bank",
                                     name=f"outps1_{n}")
                nc.tensor.matmul(
                    ops[:], iq_t[:], ar_tiles[m][:, n * FD:(n + 1) * FD],
                    start=True, stop=False,
                )
                for j in range(KT):
                    lhsT = mt_tiles[j][:, m * P:(m + 1) * P]
                    nc.tensor.matmul(
                        ops[:], lhsT, a_tiles[j][:, n * FD:(n + 1) * FD],
                        start=False, stop=(j == KT - 1),
                    )
                    if n == 0:
                        nc.tensor.matmul(
                            deg_ps[:], lhsT, ones_t[:],
                            start=(j == 0), stop=(j == KT - 1),
                        )
                if n == 0:
                    sca = emit_deg_scale(m, deg_ps)
                emit_epilogue(m, n, ops, sca)
    nc.compile()
    return nc


def _get_nc():
    if "nc" not in _CACHE:
        _CACHE["nc"] = _build_bass()
    return _CACHE["nc"]


def _make_in_maps(A_f32):
    import ml_dtypes

    bf = ml_dtypes.bfloat16
    Ab = A_f32.astype(bf)
    ATb = np.ascontiguousarray(Ab.T)

    ones = np.ones((P, 1), dtype=bf)
    iq = (0.25 * np.eye(P, dtype=np.float32)).astype(bf)
    in_maps = []
    for c in range(NCORES):
        sl = slice(c * R, (c + 1) * R)
        in_maps.append({
            "a": Ab,
            "art": np.ascontiguousarray(ATb[:, sl]),
            "ar": np.ascontiguousarray(Ab[sl, :]),
            "ones": ones,
            "iq": iq,
        })
    return in_maps


def kernel(A, w1a=None, w1b=None, w2a=None, **_unused):
    # w1a/w1b/w2a only enter the reference through a softmax over a
    # singleton axis (== 1.0), so the output does not depend on them.
    from concourse.bass_utils import run_bass_kernel_spmd

    A = np.asarray(A, dtype=np.float32)
    assert A.shape == (N, N), A.shape
    nc = _get_nc()
    in_maps = _make_in_maps(A)
    res = run_bass_kernel_spmd(nc, in_maps, core_ids=list(range(NCORES)))
    out = np.concatenate(
        [res.results[c]["out"] for c in range(NCORES)], axis=0
    )
    return out[None].astype(np.float32)
